# revision 9
# baseline (speedup 1.0000x reference)
"""Trainium2 Bass kernel for the DiscretizedDPLRSSMBlock problem.

Computes, for h, x of shape [4096, 4096] (batch, hidden):

    out = h + (h * a_diag + (h @ q_vec) @ p_vec.T) + x @ b_mat        (DELTA = 1.0)
        = h * (1 + a_diag) + (h @ q_vec) @ p_vec.T + x @ b_mat

Sharding: data-parallel over the batch axis across 8 NeuronCores (512 rows
per core); a_diag/p_vec/q_vec/b_mat replicated.

Per-core kernel works in a transposed layout (hidden on partitions):
    outT[n, m] = sum_k B[k, n] * xT[k, m]        (x @ B, B tiles are the
                                                  stationary matmul operand)
               + sum_r p[n, r] * hqT[r, m]       (rank-4 term, hqT = q^T hT)
               + (1 + a[n]) * hT[n, m]           (per-partition scalar on DVE)

The x@B and q^T@hT contractions run in fp8 e4m3 with perf_mode=DoubleRow
(2 fp8 MACs/cell/cycle; a pair of adjacent k-tiles per instruction),
optionally with the first NK_BF k-tiles of x@B in bf16 for accuracy
headroom.  e4m3 range handling: b_mat is ~2^-7 max so it carries a 2^13
pre-scale (p_vec and (1+a) carry the same scale so everything reaching
PSUM/DVE is uniformly scaled); q carries 2^9, unscaled at the PSUM->SBUF
copy of hq.  The host unscales the fp32 output by 2^-13 (exact).

All streamed inputs are pre-swizzled on the host into per-partition-
contiguous layouts ([...][128 partitions][k rows][cols]) so each HWDGE
descriptor covers a whole k-chunk row run (1-4KB) instead of 512B —
the descriptor-issue rate on the ring is the binding constraint
otherwise.  Ring assignment: Sync streams b (PE-critical), Scalar
streams x/h8 then h0/h1 then the output tiles, GpSimd (SWDGE) carries
the constants and h2/h3.
"""

import numpy as np
import ml_dtypes

import concourse.mybir as mybir
import concourse.tile as tile
from concourse import bacc
from concourse.bass_utils import run_bass_kernel_spmd

HIDDEN = 4096
BATCH = 4096
RANK = 4
N_CORES = 8
MB = BATCH // N_CORES  # 512 batch rows per core
P = 128
KT = HIDDEN // P       # 32 contraction tiles
NT = HIDDEN // P       # 32 output row tiles (hidden)
NGROUP = 4             # n-tiles per b-column streaming group (512 cols)
NG = NT // NGROUP      # 8 column groups

NK_BF = 0              # leading k-tiles computed in bf16 (accuracy dial)
KT8 = KT - NK_BF       # k-tiles computed in fp8 DoubleRow (must be even)
SCALE = 2.0 ** 13      # shared PSUM scale (b pre-scaled into e4m3 range)
SCALE_Q = 2.0 ** 9     # q pre-scale (unscaled at the hq PSUM->SBUF copy)

BF16 = mybir.dt.bfloat16
F8 = mybir.dt.float8e4
F32 = mybir.dt.float32
DR = mybir.MatmulPerfMode.DoubleRow


def _f8_chunks(kt8):
    """fp8 episode lengths: small leading chunks for a fast PE start."""
    chunks, rem = [2, 6], kt8 - 8
    while rem > 0:
        c = min(8, rem)
        chunks.append(c)
        rem -= c
    return chunks


def build_bass():
    """Build the single-core Tile program (same program runs SPMD on all 8)."""
    nc = bacc.Bacc("TRN2", target_bir_lowering=False, debug=False)

    # Host-swizzled layouts: dim order [group][partition][k-row][col] so a
    # chunk DMA is one contiguous multi-KB run per partition.
    b8s = nc.dram_tensor("b8s", [NG, P, KT8, NGROUP * P], F8, kind="ExternalInput")
    x8s = nc.dram_tensor("x8s", [P, KT8, MB], F8, kind="ExternalInput")
    if NK_BF:
        b16s = nc.dram_tensor(
            "b16s", [NG, P, NK_BF, NGROUP * P], BF16, kind="ExternalInput"
        )
        x16s = nc.dram_tensor("x16s", [P, NK_BF, MB], BF16, kind="ExternalInput")
    hs = nc.dram_tensor("hs", [P, KT, MB], BF16, kind="ExternalInput")
    h8s = nc.dram_tensor("h8s", [P, KT, MB], F8, kind="ExternalInput")
    q8s = nc.dram_tensor("q8s", [P, KT, 16], F8, kind="ExternalInput")
    pT = nc.dram_tensor("pT", [RANK, HIDDEN], BF16, kind="ExternalInput")
    a_r = nc.dram_tensor("a_r", [P, NT], F32, kind="ExternalInput")
    outT = nc.dram_tensor("outT", [HIDDEN, MB], F32, kind="ExternalOutput")

    # Episode chunking: each episode streams one (dtype, k-range) chunk of b
    # and consumes it immediately.  The optional bf16 prefix is one episode;
    # fp8 episodes start with 2+6 k-tiles so the first matmul only waits on
    # a 128KB transfer, then settle at 8 (4 DoubleRow matmuls per n-subtile).
    EPS = []
    if NK_BF:
        EPS.append(("bf", 0, NK_BF))
    t0 = 0
    for c in _f8_chunks(KT8):
        EPS.append(("f8", t0, c))
        t0 += c
    NEP = len(EPS)
    HCH = KT // 4  # hT DMA chunk (8 k-tiles)

    with (
        tile.TileContext(nc) as tc,
        tc.tile_pool(name="const", bufs=1) as cpool,
        tc.tile_pool(name="bcols", bufs=4) as bpool,
        tc.tile_pool(name="psum", bufs=6, space="PSUM") as pspool,
        tc.tile_pool(name="outs", bufs=4) as opool,
    ):
        def dma_b_ep(g, e, name):
            kind, t0, ln = EPS[e]
            if kind == "bf":
                bc = bpool.tile([P, ln, NGROUP * P], BF16, tag="bbf", name=name)
                nc.sync.dma_start(bc[:], b16s[g, :, t0 : t0 + ln, :])
            else:
                bc = bpool.tile([P, ln, NGROUP * P], F8, tag=f"b{e}", name=name)
                nc.sync.dma_start(bc[:], b8s[g, :, t0 : t0 + ln, :])
            return bc

        def dma_b_group(g):
            return [dma_b_ep(g, e, f"b{g}_{e}") for e in range(NEP)]

        xc, hc, h8c = [], [], []

        def dma_x(e):
            kind, t0, ln = EPS[e]
            if kind == "bf":
                xt = cpool.tile([P, ln, MB], BF16, tag="xbf")
                nc.scalar.dma_start(xt[:], x16s[:, t0 : t0 + ln, :])
            else:
                xt = cpool.tile([P, ln, MB], F8, tag=f"x{e}")
                nc.scalar.dma_start(xt[:], x8s[:, t0 : t0 + ln, :])
            xc.append(xt)

        def dma_h(cc, eng):
            ht = cpool.tile([P, HCH, MB], BF16, tag=f"h{cc}", name=f"h{cc}")
            eng.dma_start(ht[:], hs[:, cc * HCH : (cc + 1) * HCH, :])
            hc.append(ht)

        def dma_h8(cc):
            ht = cpool.tile([P, HCH, MB], F8, tag=f"h8_{cc}", name=f"h8_{cc}")
            nc.scalar.dma_start(ht[:], h8s[:, cc * HCH : (cc + 1) * HCH, :])
            h8c.append(ht)

        # ---- constants + late h chunks on the GpSimd (SWDGE) queue ----
        q8_sb = cpool.tile([P, KT, 16], F8, tag="q8")
        nc.gpsimd.dma_start(q8_sb[:], q8s[:])
        pT_sb = cpool.tile([P, HIDDEN], BF16, tag="pT")
        nc.any.memset(pT_sb[:], 0.0)
        nc.gpsimd.dma_start(pT_sb[0:RANK, :], pT[:, :])
        a1 = cpool.tile([P, NT], F32, tag="a1")
        nc.gpsimd.dma_start(a1[:], a_r[:, :])

        # ---- b on Sync; x/h8 interleaved on Scalar (matching PE order) ----
        bcs0 = dma_b_group(0)
        dma_x(0)
        dma_x(1)
        dma_h8(0)
        for e in range(2, NEP):
            dma_x(e)
            if e - 1 < 4:
                dma_h8(e - 1)
        for cc in range(max(1, NEP - 1), 4):
            dma_h8(cc)
        dma_h(0, nc.scalar)
        dma_h(1, nc.scalar)
        dma_h(2, nc.gpsimd)
        dma_h(3, nc.gpsimd)

        def sub_epilogue(tn, ps):
            ot = opool.tile([P, MB], F32, tag="ot", name=f"ot{tn}")
            nc.vector.scalar_tensor_tensor(
                ot[:],
                hc[tn // HCH][:, tn % HCH],
                a1[:, tn : tn + 1],
                ps[:],
                mybir.AluOpType.mult,
                mybir.AluOpType.add,
            )
            nc.scalar.dma_start(outT[tn * P : (tn + 1) * P, :], ot[:])

        def rank4(tn, ps):
            nc.tensor.matmul(
                ps[:],
                pT_sb[:, tn * P : (tn + 1) * P],
                hq_sb[:],
                start=False,
                stop=True,
            )

        def ep_matmuls(e, bc, pss, g=None):
            """Issue one episode; on the final episode (g given) inline each
            n-subtile's rank-4 + epilogue right after its last k-chunk."""
            kind, t0, ln = EPS[e]
            for sub in range(NGROUP):
                if kind == "bf":
                    for tt in range(ln):
                        nc.tensor.matmul(
                            pss[sub][:],
                            bc[:, tt, sub * P : (sub + 1) * P],
                            xc[e][:, tt],
                            start=(e == 0 and tt == 0),
                            stop=False,
                        )
                else:
                    for u in range(ln // 2):
                        nc.tensor.matmul(
                            pss[sub][:],
                            bc[:, 2 * u : 2 * u + 2, sub * P : (sub + 1) * P],
                            xc[e][:, 2 * u : 2 * u + 2],
                            start=(e == 0 and u == 0),
                            stop=False,
                            perf_mode=DR,
                        )
                if g is not None:
                    tn = g * NGROUP + sub
                    rank4(tn, pss[sub])
                    sub_epilogue(tn, pss[sub])

        # ---- group 0: mains with the hq prologue (hqT = q^T @ hT, [4,512])
        # interleaved between episodes as each h8 chunk lands; hq completes
        # before the last episode so every group gets the inline tail. ----
        pss0 = [
            pspool.tile([P, MB], F32, tag="ps", name=f"ps0_{i}")
            for i in range(NGROUP)
        ]
        hq_ps = pspool.tile([RANK, MB], F32, tag="hq", bufs=1)

        def hq_chunk(cc):
            for u in range(HCH // 2):
                nc.tensor.matmul(
                    hq_ps[:],
                    q8_sb[:, cc * HCH + 2 * u : cc * HCH + 2 * u + 2, 0:RANK],
                    h8c[cc][:, 2 * u : 2 * u + 2],
                    start=(cc == 0 and u == 0),
                    stop=(cc == 3 and u == HCH // 2 - 1),
                    perf_mode=DR,
                )

        hq_sb = cpool.tile([P, MB], BF16, tag="hq_sb")
        nc.any.memset(hq_sb[:], 0.0)

        ep_matmuls(0, bcs0[0], pss0)
        ep_matmuls(1, bcs0[1], pss0)
        hq_chunk(0)
        for e in range(2, NEP - 1):
            ep_matmuls(e, bcs0[e], pss0)
            if e - 1 < 4:
                hq_chunk(e - 1)
        for cc in range(max(1, NEP - 2), 4):
            hq_chunk(cc)
        nc.vector.tensor_scalar_mul(hq_sb[0:RANK, :], hq_ps[:], 1.0 / SCALE_Q)
        ep_matmuls(NEP - 1, bcs0[NEP - 1], pss0, g=0)

        # ---- groups 1..7 ----
        for g in range(1, NG):
            bcs = dma_b_group(g)
            pss = [
                pspool.tile([P, MB], F32, tag="ps", name=f"ps{g}_{i}")
                for i in range(NGROUP)
            ]
            for e in range(NEP):
                ep_matmuls(e, bcs[e], pss, g=g if e == NEP - 1 else None)

    nc.compile()
    return nc


_NC_CACHE = []


def _get_nc():
    if not _NC_CACHE:
        _NC_CACHE.append(build_bass())
    return _NC_CACHE[0]


LAST_RESULTS = []  # stash of the last BassKernelResults, for test harnesses


def _swz(a, kt):
    """[kt*128, cols] -> per-partition-contiguous [128, kt, cols]."""
    return np.ascontiguousarray(
        a.reshape(kt, P, a.shape[1]).transpose(1, 0, 2)
    )


def make_in_maps(h, x, a_diag, p_vec, q_vec, b_mat):
    """Shard + quantize + swizzle the full inputs into per-core in_maps."""
    h = np.asarray(h, dtype=np.float32)
    x = np.asarray(x, dtype=np.float32)
    a_diag = np.asarray(a_diag, dtype=np.float32)
    p_vec = np.asarray(p_vec, dtype=np.float32)
    q_vec = np.asarray(q_vec, dtype=np.float32)
    b_mat = np.asarray(b_mat, dtype=np.float32)

    bf = ml_dtypes.bfloat16
    f8 = ml_dtypes.float8_e4m3
    ksplit = NK_BF * P
    bs = b_mat * np.float32(SCALE)          # max |b|*2^13 ~ 128, e4m3 normal
    b8 = np.clip(bs[ksplit:], -240, 240).astype(f8)
    # b8s[g, p, t, n'] = b8[t*128+p, g*512+n']
    b8s = np.ascontiguousarray(
        b8.reshape(KT8, P, NG, NGROUP * P).transpose(2, 1, 0, 3)
    )
    q8f = np.zeros((HIDDEN, 16), dtype=f8)
    q8f[:, :RANK] = np.clip(
        q_vec * np.float32(SCALE_Q), -240, 240
    ).astype(f8)
    q8s_np = _swz(q8f, KT)
    pT_bf = np.ascontiguousarray((p_vec.T * np.float32(SCALE)).astype(bf))
    # a_r[p, t] = (1 + a_diag[t*128 + p]) * SCALE
    a_r = np.ascontiguousarray(
        ((1.0 + a_diag) * np.float32(SCALE)).reshape(NT, P).T
    ).astype(np.float32)
    if NK_BF:
        b16s = np.ascontiguousarray(
            bs[:ksplit].astype(bf)
            .reshape(NK_BF, P, NG, NGROUP * P).transpose(2, 1, 0, 3)
        )

    in_maps = []
    for c in range(N_CORES):
        sl = slice(c * MB, (c + 1) * MB)
        xT = np.ascontiguousarray(x[sl].T)
        hTc = np.ascontiguousarray(h[sl].T)
        m = {
            "b8s": b8s,
            "x8s": _swz(np.clip(xT[ksplit:], -240, 240).astype(f8), KT8),
            "hs": _swz(hTc.astype(bf), KT),
            "h8s": _swz(np.clip(hTc, -240, 240).astype(f8), KT),
            "q8s": q8s_np,
            "pT": pT_bf,
            "a_r": a_r,
        }
        if NK_BF:
            m["b16s"] = b16s
            m["x16s"] = _swz(xT[:ksplit].astype(bf), NK_BF)
        in_maps.append(m)
    return in_maps


def _axon_device_reset():
    """Best-effort heal of a wedged axon-tunneled device (NRT_EXEC_UNIT_
    UNRECOVERABLE). No-op when the axon .so isn't present."""
    try:
        import ctypes

        lib = ctypes.CDLL("/opt/axon/libaxon_pjrt.so")
        lib.axon_reset.restype = ctypes.c_int64
        lib.axon_reset()
    except Exception:
        pass


def kernel(h, x, a_diag, p_vec, q_vec, b_mat, trace=False):
    nc = _get_nc()
    in_maps = make_in_maps(h, x, a_diag, p_vec, q_vec, b_mat)
    try:
        res = run_bass_kernel_spmd(
            nc, in_maps, core_ids=list(range(N_CORES)), trace=trace
        )
    except Exception as e:
        if "UNRECOVERABLE" not in str(e) and "UNAVAILABLE" not in str(e):
            raise
        _axon_device_reset()
        res = run_bass_kernel_spmd(
            nc, in_maps, core_ids=list(range(N_CORES)), trace=trace
        )
    LAST_RESULTS.clear()
    LAST_RESULTS.append(res)

    inv = np.float32(1.0 / SCALE)
    out = np.empty((BATCH, HIDDEN), dtype=np.float32)
    for c in range(N_CORES):
        out[c * MB : (c + 1) * MB, :] = res.results[c]["outT"].T * inv
    return out
